# revision 3
# baseline (speedup 1.0000x reference)
"""CRC24A encoder (nn_CRCEncoder) as a Bass/Tile kernel on 8 Trainium2 NeuronCores.

Computation (per the reference):
    out = concat([X, (X @ G) mod 2], axis=-1)
with X [16384, 4096] of {0,1} float32 and G [4096, 24] of {0,1} float32.

Pure data parallel over the batch: each core round-trips a [2048, 4096]
shard (~64 MiB of HBM traffic), so the kernel lives at the DMA roofline
and everything else must hide under it. Design, in order of measured
impact:

  - Both HWDGE rings carry both directions: loads alternate sync/scalar
    per 4 MiB double-tile and stores alternate the other way. A single
    ring tops out ~313-350 GB/s; mixed dual-ring traffic sustains ~347
    GB/s against the ~358 GB/s per-core HBM limit (-13 us vs dedicated
    rings).
  - Row r of a group maps to (partition, slot) = (r // 2, r % 2) so each
    DMA descriptor covers consecutive DRAM rows: 33 KB store descriptors
    and per-SDMA-engine fully contiguous spans (-13 us vs the
    partition-major map).
  - Parity lands in the staging tile's last 24 columns, so each output
    group leaves in one contiguous DMA.
  - The timing repeat loop unrolls 8 passes per hardware-loop iteration;
    For_i runs an all-engine barrier in its reset block, so unrolling
    lets consecutive passes pipeline and amortizes the ramp/drain.
  - PE work runs in float32r: 128x128 chunks transpose through PSUM
    (evac copies round to f32r, alternating DVE/ACT to keep the matmul
    feed path wide), then each K-chunk is one 256-wide fp32r matmul
    (1 cycle/row vs 4 for f32) accumulating (X @ G).T for both row-tiles
    of a group at once. All values are {0,1} and sums < 2^11, so the
    reduced-mantissa fp32r path is exact. PE busy drops ~2x vs f32,
    keeping compute fully hidden under DMA.
  - Mod-2 on the exact integer sums: cast i32, AND 1, cast back; the
    [24, 256] parity block transposes back to row-major on the PE.

Measured: 204.6 us/pass steady-state (repeat-loop delta, 8 cores), vs a
~194 us pure-DMA ceiling for the same byte pattern and ~188 us at the
358 GB/s HBM spec.
"""

import contextlib

import numpy as np

import concourse.mybir as mybir
from concourse import bacc
from concourse.bass_utils import run_bass_kernel_spmd
from concourse.masks import make_identity
from concourse.tile import TileContext

N_CORES = 8
BATCH = 16384
K = 4096
CRC = 24
B_SHARD = BATCH // N_CORES  # 2048 rows per core
P = 128
N_TILES = B_SHARD // P  # 16 row-tiles per core
N_CHUNKS = K // P  # 32 K-chunks
TGROUP = 2  # row-tiles per DMA double-tile
CGROUP = 2  # transposes batched per PSUM tile
FP32 = mybir.dt.float32
F32R = mybir.dt.float32r
I32 = mybir.dt.int32


def _crc_body(
    tc,
    o_d,
    x_d,
    g_d,
    repeats,
    tgroup=TGROUP,
    cgroup=CGROUP,
    x_bufs=4,
    xt_bufs=4,
    pst_bufs=4,
    pp_bufs=2,
    tp_bufs=2,
    copy_mode="alt",  # "alt" | "dve" | "act"
    schedule=None,
    unroll=16,
    staggered=False,
    rings="mix",  # "split": loads sync / stores scalar; "mix": alternate
    layout="pt",  # row <-> (partition, slot) map; "pt" gives 33 KB store descs
    store_lag=0,  # emit group g's store after group g+lag's load (hides the
    # parity wait so the ring FIFO head is never blocked)
    pgroup=0,  # >0: phase DMA into read bursts / write bursts of this many
    # groups (both rings same direction at once; fewer HBM R/W turnarounds)
):
    nc = tc.nc
    if schedule is None:
        schedule = [tgroup] * (N_TILES // tgroup)
    assert sum(schedule) == N_TILES
    with contextlib.ExitStack() as stk:
        consts = stk.enter_context(tc.tile_pool(name="consts", bufs=1))
        xpool = stk.enter_context(tc.tile_pool(name="x", bufs=x_bufs))
        xtpool = stk.enter_context(tc.tile_pool(name="xt", bufs=xt_bufs))
        pstpool = stk.enter_context(
            tc.tile_pool(name="pst", bufs=pst_bufs, space="PSUM")
        )
        pppool = stk.enter_context(tc.tile_pool(name="ppar", bufs=pp_bufs, space="PSUM"))
        tppool = stk.enter_context(tc.tile_pool(name="tpar", bufs=tp_bufs, space="PSUM"))
        tpsbpool = stk.enter_context(tc.tile_pool(name="tpsb", bufs=2))
        paripool = stk.enter_context(tc.tile_pool(name="pari", bufs=2))

        ident = consts.tile([P, P], FP32)
        make_identity(nc, ident)
        ident24 = consts.tile([CRC, CRC], FP32)
        make_identity(nc, ident24)
        # G chunk c ([128, 24] rows c*128..(c+1)*128) lives at columns
        # [c*24, (c+1)*24) so each matmul's stationary lhsT is a contiguous
        # 24-column slice.
        g_sb = consts.tile([P, N_CHUNKS * CRC], F32R)
        if g_d.shape == [P, N_CHUNKS * CRC]:
            nc.scalar.dma_start(out=g_sb, in_=g_d.bitcast(F32R))
        else:
            nc.scalar.dma_start(
                out=g_sb.rearrange("p (c m) -> p c m", m=CRC),
                in_=g_d.bitcast(F32R).rearrange("(c p) m -> p c m", p=P),
            )

        if copy_mode == "alt":
            copy_engines = [nc.vector.tensor_copy, nc.scalar.copy]
        elif copy_mode == "dve":
            copy_engines = [nc.vector.tensor_copy]
        else:
            copy_engines = [nc.scalar.copy]

        if rings == "split":
            load_engines, store_engines = [nc.sync], [nc.scalar]
        else:
            load_engines = [nc.sync, nc.scalar]
            store_engines = [nc.scalar, nc.sync]

        def rearr(ap, rows, tg):
            if layout == "tp":
                return ap[rows, :].rearrange("(two p) k -> p two k", p=P)
            return ap[rows, :].rearrange("(p two) k -> p two k", two=tg)

        pending_stores = []

        def emit_store(gi, tg, rows, x2):
            store_engines[gi % len(store_engines)].dma_start(
                out=rearr(o_d, rows, tg),
                in_=x2,
            )

        def one_pass():
            n_copies = 0
            row0 = 0
            emitted = []

            def do_load(gi, tg):
                nonlocal row0
                rows = slice(row0 * P, (row0 + tg) * P)
                row0 += tg
                x2 = xpool.tile([P, tg, K + CRC], FP32, tag="x2")
                load_engines[gi % len(load_engines)].dma_start(
                    out=x2[:, :, 0:K],
                    in_=rearr(x_d, rows, tg),
                )
                return rows, x2

            def do_compute(gi, tg, rows, x2):
                nonlocal n_copies
                # Both row-tiles accumulate into one transposed parity tile
                # ppT [24, tg*128] = (Xg @ G).T, built from 256-wide fp32r
                # matmuls (full PE rate; moving dim >= 256).
                ppT = pppool.tile([CRC, tg * P], FP32, tag="ppT")
                for g in range(N_CHUNKS // cgroup):
                    pst = pstpool.tile([P, cgroup, tg, P], FP32, tag="pst")
                    for j in range(cgroup):
                        c = g * cgroup + j
                        for t in range(tg):
                            nc.tensor.transpose(
                                pst[:, j, t], x2[:, t, c * P : (c + 1) * P], ident
                            )
                    xt = xtpool.tile([P, cgroup, tg, P], F32R, tag="xt")
                    copy_engines[n_copies % len(copy_engines)](xt, pst)
                    n_copies += 1
                    for j in range(cgroup):
                        c = g * cgroup + j
                        nc.tensor.matmul(
                            ppT,
                            g_sb[:, c * CRC : (c + 1) * CRC],
                            xt[:, j],
                            start=(c == 0),
                            stop=(c == N_CHUNKS - 1),
                        )
                # Evacuate [24, tg*128], transpose each tile's half back on
                # the PE, then mod-2 of exact-integer f32 sums: cast i32,
                # AND 1, cast back.
                tpsb = tpsbpool.tile([CRC, tg * P], FP32, tag="tpsb")
                nc.vector.tensor_copy(tpsb, ppT)
                for t in range(tg):
                    tp = tppool.tile([P, CRC], FP32, tag="tp")
                    nc.tensor.transpose(tp, tpsb[:, t * P : (t + 1) * P], ident24)
                    pari = paripool.tile([P, CRC], I32, tag="pari")
                    nc.vector.tensor_copy(pari, tp)
                    nc.vector.tensor_scalar(
                        pari, pari, 1, None, mybir.AluOpType.bitwise_and
                    )
                    nc.vector.tensor_copy(x2[:, t, K : K + CRC], pari)

            if pgroup:
                n_ph, r_ph = divmod(len(schedule), pgroup)
                assert r_ph == 0
                gi = 0
                for ph in range(n_ph):
                    batch = []
                    for _ in range(pgroup):
                        tg = schedule[gi]
                        rows, x2 = do_load(gi, tg)
                        batch.append((gi, tg, rows, x2))
                        gi += 1
                    for args in batch:
                        do_compute(*args)
                    for args in batch:
                        emit_store(args[0], args[1], args[2], args[3])
            else:
                for gi, tg in enumerate(schedule):
                    rows, x2 = do_load(gi, tg)
                    do_compute(gi, tg, rows, x2)
                    pending_stores.append((gi, tg, rows, x2))
                    if len(pending_stores) > store_lag:
                        emit_store(*pending_stores.pop(0))

        def flush_stores():
            while pending_stores:
                emit_store(*pending_stores.pop(0))

        if repeats == 1:
            one_pass()
            flush_stores()
        else:
            # The For_i reset block runs an all-engine barrier each
            # iteration, serializing the pass ramp/drain. Unrolling U
            # passes per iteration lets consecutive passes pipeline inside
            # the body and amortizes the barrier U-fold.
            n_full, rem = divmod(repeats, unroll)
            if n_full:
                with tc.For_i(0, n_full, 1, staggered_reset=staggered):
                    for _ in range(unroll):
                        one_pass()
                    flush_stores()
            for _ in range(rem):
                one_pass()
            flush_stores()


def pack_g(g_mat: np.ndarray) -> np.ndarray:
    """[4096, 24] -> chunk-major [128, 32*24]: chunk c's rows land in columns
    [c*24, (c+1)*24), row c*128+p on partition p."""
    return np.ascontiguousarray(
        g_mat.reshape(N_CHUNKS, P, CRC).transpose(1, 0, 2).reshape(P, N_CHUNKS * CRC)
    )


def build_crc_module(repeats: int = 1, **body_kwargs):
    nc = bacc.Bacc(
        "TRN2", target_bir_lowering=False, debug=False, num_devices=N_CORES
    )
    x_d = nc.dram_tensor("inputs", [B_SHARD, K], FP32, kind="ExternalInput").ap()
    g_d = nc.dram_tensor(
        "g_packed", [P, N_CHUNKS * CRC], FP32, kind="ExternalInput"
    ).ap()
    o_d = nc.dram_tensor("out", [B_SHARD, K + CRC], FP32, kind="ExternalOutput").ap()
    with TileContext(nc) as tc:
        _crc_body(tc, o_d, x_d, g_d, repeats, **body_kwargs)
    nc.compile()
    return nc


_NC_CACHE = None


def kernel(inputs: np.ndarray, g_mat: np.ndarray) -> np.ndarray:
    global _NC_CACHE
    if _NC_CACHE is None:
        _NC_CACHE = build_crc_module(repeats=1)
    nc = _NC_CACHE

    x = np.ascontiguousarray(np.asarray(inputs, dtype=np.float32))
    g = np.ascontiguousarray(np.asarray(g_mat, dtype=np.float32))
    assert x.shape == (BATCH, K) and g.shape == (K, CRC)
    gp = pack_g(g)

    in_maps = [
        {"inputs": x[i * B_SHARD : (i + 1) * B_SHARD], "g_packed": gp}
        for i in range(N_CORES)
    ]
    res = run_bass_kernel_spmd(nc, in_maps, core_ids=list(range(N_CORES)))
    out = np.concatenate([r["out"] for r in res.results], axis=0)
    return out.astype(np.float32, copy=False)
